# revision 21
# baseline (speedup 1.0000x reference)
"""Trainium2 Bass kernel for CRATE-style subspace attention (nn_Attention_37091337568712).

Reference computation (fp32):
    w = x @ Wqkv                    # (b, n, 1024), shared q=k=v projection
    w -> (b, h=16, n, d=64)
    S = (w @ w^T) * d^-0.5          # per head, (b, h, n, n)
    attn = softmax(S, axis=-1) * (1 - mask[:, None, None, :])
    out = attn @ w                  # (b, h, n, d)
    y = out.reshape(b, n, 1024) @ Wout + bout

Sharding: 8 cores = 2 batches x 4 head-groups (4 heads each). Each core
computes its 4 heads end-to-end including a partial output projection;
host sums the 4 partials per batch and adds bout.

Design highlights (vs the fp32r baseline, 346us -> this):
  - bf16 everywhere on the PE: xT/Wqkv/Wout shipped as bf16 from the host,
    wT2/V'/E/osT bf16. End-to-end numeric sim: rel err ~5.9e-3 (gate 2e-2).
  - exp split across two engines: ACT computes true exp for 36/64 tiles per
    block, DVE computes a Schraudolph bit-trick exp (int16 out = bf16 bit
    pattern of 2^(a*s+b); the DVE fp32->int16 store rounds-to-nearest) for
    the rest. This halves the single-engine ACT exp bottleneck.
  - pipelined attention: [128,512] S tiles double-buffered per head, AV
    matmuls lag S by ~2 steps so the PE never waits on the exp engines.
  - softmax denominator via an unmasked ones column in the AV stationary
    operand (M=65): row 64 of the AV accumulator = sum_j e[j, i].
  - V' (masked w^T) built by a single batched XBAR DMA transpose per pair.
  - 1/den broadcast along partitions with a stride-0-partition DMA (no PE,
    no PSUM); scales run mid-attention so the output projection can start
    immediately after the last AV matmul, keeping the PE warm in the tail.
"""

import sys

if "/opt/trn_rl_repo" not in sys.path:
    sys.path.insert(0, "/opt/trn_rl_repo")

import numpy as np
import ml_dtypes

import concourse.bass as bass
import concourse.mybir as mybir
from concourse import masks
from concourse.bass_utils import run_bass_kernel_spmd
from concourse.tile import TileContext

FP = mybir.dt.float32
I32 = mybir.dt.int32
I16 = mybir.dt.int16
BF = mybir.dt.bfloat16
F32R = mybir.dt.float32r
BF_NP = ml_dtypes.bfloat16


def _split_multiwaits(bir_json: bytes) -> bytes:
    """This container's walrus supports a single sync wait per instruction
    (setupSyncWait: 'Too many sync wait commands'). Split any multi-wait
    instruction into a chain of single-wait EventSemaphore instructions
    (same engine, program order) followed by the original instruction
    keeping its last wait."""
    import json

    bir = json.loads(bir_json)
    changed = False
    for fn in bir.get("functions", []):
        for bb in fn.get("blocks", []):
            insts = bb.get("instructions")
            if insts is None:
                continue
            new_insts = []
            for ins in insts:
                si = ins.get("sync_info")
                waits = si.get("on_wait") if si else None
                if waits and len(waits) > 1:
                    changed = True
                    for wi, w in enumerate(waits[:-1]):
                        new_insts.append({
                            "name": f"{ins['name']}_w{wi}",
                            "opcode": "EventSemaphore",
                            "engine": ins["engine"],
                            "ins": [],
                            "outs": [],
                            "debug": ins.get("debug", 0),
                            "sync_info": {"on_wait": [w], "on_update": []},
                        })
                    si["on_wait"] = [waits[-1]]
                new_insts.append(ins)
            bb["instructions"] = new_insts
    if not changed:
        return bir_json
    return json.dumps(bir).encode()


def _install_bir_legalizer():
    from concourse import bass2jax, bass_utils

    if getattr(bass2jax, "_multiwait_legalizer_installed", False):
        return
    orig = bass_utils.compile_bir_kernel

    def wrapped(bir_json, tmpdir, neff_name="file.neff"):
        try:
            return orig(_split_multiwaits(bytes(bir_json)), tmpdir, neff_name)
        except BaseException as e:
            # XLA swallows python exceptions from the compile callback;
            # persist the real error for debugging.
            import subprocess, traceback
            try:
                with open("/tmp/bass_compile_err.txt", "w") as f:
                    traceback.print_exc(file=f)
                    ee = e
                    while ee is not None:
                        if isinstance(ee, subprocess.CalledProcessError):
                            out = ee.stdout or ""
                            if isinstance(out, bytes):
                                out = out.decode(errors="replace")
                            f.write("\n==WALRUS STDOUT (tail)==\n" + out[-12000:])
                        ee = ee.__cause__ or ee.__context__
            except Exception:
                pass
            raise

    bass2jax.compile_bir_kernel = wrapped
    bass2jax._multiwait_legalizer_installed = True


N = 2048          # sequence length
DIM = 1024        # model dim
DH = 64           # head dim
EC = 256          # local inner columns (4 heads)
KC = DIM // 128   # 8 contraction chunks for the projection
JC = N // 128     # 16 key chunks
PAIRS = 2         # head pairs per core (2 heads stacked on 128 partitions)
SCALE = DH ** -0.5

# Schraudolph exp for bf16 bit patterns: rint(a*s + b) as int16 == bf16 bits
# of approx exp(SCALE*s). c tuned in numpy sim against the full pipeline.
SCHR_C = 0.0434
A_EXP = 128.0 / np.log(2.0) * SCALE
B_EXP = 128.0 * (127.0 - SCHR_C)

EXPF = mybir.ActivationFunctionType.Exp

_program_cache = {}


def _act_tile(jc, sb, hh):
    """exp engine assignment per [128,512] tile: ACT 36/64 per block."""
    return hh == 0 or jc in (0, 8)


def build_program():
    nc = bass.Bass()

    xT = nc.declare_dram_parameter("xT", [DIM, N], BF, isOutput=False)
    wqkv = nc.declare_dram_parameter("wqkv", [DIM, EC], BF, isOutput=False)
    wout = nc.declare_dram_parameter("wout", [EC, DIM], BF, isOutput=False)
    mask_d = nc.declare_dram_parameter("mask", [N], I32, isOutput=False)
    y = nc.declare_dram_parameter("y", [N, DIM], FP, isOutput=True)
    # DRAM bounce for the 1/den rows: DRAM-side APs allow stride-0 repeat
    # reads, which SBUF partition dims do not
    dscr = nc.declare_dram_parameter("dscr", [2, PAIRS, 2, N // 2], FP,
                                     isOutput=True)

    with TileContext(nc) as tc:
        with (
            tc.tile_pool(name="const", bufs=1) as constp,
            tc.tile_pool(name="wts", bufs=1) as wts,
            tc.tile_pool(name="persist", bufs=1) as persist,
            tc.tile_pool(name="xin", bufs=8) as xin,
            tc.tile_pool(name="epool", bufs=8) as epool,
            tc.tile_pool(name="bsb", bufs=2) as bsb,
            tc.tile_pool(name="trpool", bufs=2) as trpool,
            tc.tile_pool(name="ysb", bufs=3) as ysbp,
        ):
            # ---- weights + input stream first so the projection starts ASAP
            wq_sb = wts.tile([128, KC, EC], BF)
            nc.sync.dma_start(wq_sb[:], wqkv.rearrange("(kc p) e -> p kc e", p=128))
            xts = []
            for kc in range(KC):
                xt = xin.tile([128, N], BF, name="xt")
                nc.sync.dma_start(xt[:], xT[kc * 128:(kc + 1) * 128, :])
                xts.append(xt)
            wout_sb = wts.tile([128, PAIRS, DIM], BF)
            nc.scalar.dma_start(wout_sb[:], wout.rearrange("(pc p) m -> p pc m", p=128))

            # ---- constants / small inputs (second DMA queue) ----
            ident_f = constp.tile([16, 16], FP)
            masks.make_identity(nc, ident_f[:])
            mask_i = constp.tile([16, 128], I32)
            nc.scalar.dma_start(mask_i[:], mask_d.rearrange("(a b) -> a b", a=16))
            mask_f = constp.tile([16, 128], FP)
            # 1 - mask, cast int32 -> fp32
            nc.vector.tensor_scalar(
                out=mask_f[:], in0=mask_i[:], scalar1=-1.0, scalar2=1.0,
                op0=mybir.AluOpType.mult, op1=mybir.AluOpType.add,
            )

            # ---- persistent big tiles ----
            wT2 = persist.tile([128, PAIRS, N], BF)       # [d2, pair, i]
            v2 = persist.tile([128, PAIRS, JC, 130], BF)  # [j, pair, jc, d2+ones]
            raw2 = persist.tile([128, PAIRS, N], F32R)    # unscaled attn out
            osT2 = persist.tile([128, PAIRS, N], BF)      # scaled attn out
            maskc = persist.tile([128, JC], FP)           # (1-mask) in [j%128, jc]
            den_sp = persist.tile([128, 64], FP)          # spread denominators
            recip_sp = persist.tile([128, 64], FP)
            bcast = persist.tile([128, PAIRS, 2, N // 2], FP)  # 1/den bcast tiles

            # ---- phase 1: projection  wT2[d2, i] = Wqkv_cols^T @ x^T ----
            with tc.tile_pool(name="ps_proj", bufs=1, space="PSUM") as ps_proj:
                proj_ps = [ps_proj.tile([128, 1024], FP, name=f"proj{t}", tag=f"proj{t}")
                           for t in range(4)]
                for kc in range(KC):
                    for pair in range(PAIRS):
                        for nb in range(2):
                            for sb in range(2):
                                nc.tensor.matmul(
                                    proj_ps[pair * 2 + nb][:, sb * 512:(sb + 1) * 512],
                                    wq_sb[:, kc, pair * 128:(pair + 1) * 128],
                                    xts[kc][:, nb * 1024 + sb * 512:
                                            nb * 1024 + (sb + 1) * 512],
                                    start=(kc == 0), stop=(kc == KC - 1),
                                )
                for pair in range(PAIRS):
                    for nb in range(2):
                        nc.scalar.copy(
                            wT2[:, pair, nb * 1024:(nb + 1) * 1024],
                            proj_ps[pair * 2 + nb][:],
                        )

            # ---- phase 2: mask layout + V' via DMA xbar transposes ----
            with tc.tile_pool(name="ps_tr", bufs=1, space="PSUM") as ps_tr:
                mt_ps = ps_tr.tile([128, 16], FP, tag="trm")
                nc.tensor.transpose(mt_ps[:], mask_f[:], ident_f[:])
                nc.vector.tensor_copy(maskc[:], mt_ps[:])

            # unmasked ones columns (64 and 129): the AV matmul's M=65
            # weight includes them so row 64 of the AV accumulator becomes
            # the (unmasked) softmax denominator for free.
            nc.vector.memset(v2[:, :, :, 64:130:65], 1.0)
            for pair in range(PAIRS):
                # all 16 per-jc [128,128] transposes of this pair in ONE
                # xbar DMA: out[:, jc, :] = wT2[:, pair, jc*128:+128].T
                tr2 = trpool.tile([128, JC, 128], BF, name="tr2", tag="tr2")
                nc.sync.dma_start_transpose(tr2[:], wT2[:, pair, :])
                for jc in range(JC):
                    # V' = (1 - mask_j) * w_j for both heads in one op;
                    # alternate engines (ACT does copy-with-scale-AP)
                    vdst = v2[:, pair, jc, 0:130].rearrange(
                        "p (h x) -> p h x", h=2)[:, :, 0:64]
                    vsrc = tr2[:, jc, :].rearrange("p (h x) -> p h x", h=2)
                    nc.vector.tensor_scalar_mul(vdst, vsrc,
                                                maskc[:, jc:jc + 1])

            # ---- helper emitters used inside the attention loop ----
            def emit_block_finish(pair, ibh, av_t):
                """den rows -> spread slots, raw evac, recip, bcast DMA."""
                i0 = ibh * 1024
                k0 = (pair * 2 + ibh) * 2
                for hh in range(2):
                    k = k0 + hh
                    trow = bsb.tile([1, 1024], FP, name=f"trow{hh}",
                                    tag=f"trow{hh}")
                    if hh == 0:
                        nc.scalar.copy(trow[:], av_t[hh][64:65, :])
                        nc.scalar.copy(
                            raw2[0:64, pair, i0:i0 + 1024],
                            av_t[hh][0:64, :],
                        )
                    else:
                        nc.vector.tensor_copy(trow[:], av_t[hh][64:65, :])
                        nc.vector.tensor_copy(
                            raw2[64:128, pair, i0:i0 + 1024],
                            av_t[hh][0:64, :],
                        )
                    # [1, 1024] -> [128, 8] spread (DMA iterates the out AP
                    # partition-major)
                    nc.sync.dma_start(den_sp[:, k * 8:(k + 1) * 8], trow[:])
                # this block's denominators are complete: reciprocal now
                nc.vector.reciprocal(
                    recip_sp[:, k0 * 8:(k0 + 2) * 8],
                    den_sp[:, k0 * 8:(k0 + 2) * 8],
                )
                # gather each head's recip into a DRAM row, then broadcast
                # it down 64 partitions with a stride-0 repeat-read DMA
                # (no PE, no PSUM)
                for hh in range(2):
                    k = k0 + hh
                    nc.sync.dma_start(dscr[hh, pair, ibh, :],
                                      recip_sp[:, k * 8:(k + 1) * 8])
                    nc.sync.dma_start(
                        bcast[hh * 64:(hh + 1) * 64, pair, ibh, :],
                        dscr[hh, pair, ibh, :].partition_broadcast(64),
                    )

            def emit_scale(pair, ibh):
                """osT2 = raw2 * bcast for one (pair, i-half) on DVE."""
                i0 = ibh * 1024
                for hh in range(2):
                    p0 = hh * 64
                    nc.vector.tensor_tensor(
                        out=osT2[p0:p0 + 64, pair, i0:i0 + 1024],
                        in0=raw2[p0:p0 + 64, pair, i0:i0 + 1024].bitcast(FP),
                        in1=bcast[p0:p0 + 64, pair, ibh, :],
                        op=mybir.AluOpType.mult,
                    )

            # ---- phase 3: attention ----
            # Blocks: (pair, i-half). Per (jc, sb): two row-packed S matmuls,
            # two exp tiles (ACT or DVE), and lagged AV matmuls so the PE
            # stays ahead of the exp engines. Scales for finished blocks are
            # interleaved into the next block's stream.
            with (
                tc.tile_pool(name="ps_s", bufs=2, space="PSUM") as ps_s,
                tc.tile_pool(name="ps_av", bufs=1, space="PSUM") as ps_av,
            ):
                AV_LAG = 4
                for pair in range(PAIRS):
                    for ibh in range(2):
                        i0 = ibh * 1024
                        blk = pair * 2 + ibh
                        av_t = [ps_av.tile([65, 1024], FP, name=f"av{hh}", tag=f"av{hh}")
                                for hh in range(2)]
                        pend = []

                        def flush_av(av_t=av_t, pair=pair, pend=pend):
                            jc, sb, e_t = pend.pop(0)
                            for hh in range(2):
                                nc.tensor.matmul(
                                    av_t[hh][:, sb * 512:(sb + 1) * 512],
                                    v2[:, pair, jc, hh * 65:hh * 65 + 65],
                                    e_t[hh][:],
                                    start=(jc == 0),
                                    stop=(jc == JC - 1),
                                )

                        for jc in range(JC):
                            # previous block's scale lands mid-stream once its
                            # bcast tiles are ready
                            if jc == 6 and blk > 0:
                                emit_scale((blk - 1) // 2, (blk - 1) % 2)
                            for sb in range(2):
                                s_t = [ps_s.tile([128, 512], FP, name=f"s{hh}",
                                                 tag=f"s{hh}") for hh in range(2)]
                                for hh in range(2):
                                    p0 = hh * 64
                                    nc.tensor.matmul(
                                        s_t[hh][:],
                                        wT2[p0:p0 + 64, pair, jc * 128:(jc + 1) * 128],
                                        wT2[p0:p0 + 64, pair,
                                            i0 + sb * 512:i0 + (sb + 1) * 512],
                                        start=True, stop=True,
                                        tile_position=(p0, 0),
                                    )
                                e_t = []
                                for hh in range(2):
                                    e = epool.tile([128, 512], BF, name=f"e{hh}",
                                                   tag=f"e{hh}")
                                    if _act_tile(jc, sb, hh):
                                        nc.scalar.activation(e[:], s_t[hh][:], EXPF,
                                                             scale=SCALE)
                                    else:
                                        nc.vector.tensor_scalar(
                                            out=e[:].bitcast(I16), in0=s_t[hh][:],
                                            scalar1=float(A_EXP), scalar2=float(B_EXP),
                                            op0=mybir.AluOpType.mult,
                                            op1=mybir.AluOpType.add,
                                        )
                                    e_t.append(e)
                                pend.append((jc, sb, e_t))
                                if len(pend) > AV_LAG:
                                    flush_av()
                        while pend:
                            flush_av()
                        emit_block_finish(pair, ibh, av_t)

            # ---- tail: last scale + output projection ----
            # outproj PSUM reuses the banks the attention pools vacated; the
            # WAR dependencies Tile inserts cover the handoff.
            with tc.tile_pool(name="ps_y", bufs=2, space="PSUM") as ps_y:
                emit_scale(1, 1)
                for ic in range(16):
                    y_ps = ps_y.tile([128, 1024], FP, name="yp", tag="y")
                    for sb in range(2):
                        for pair in range(PAIRS):
                            nc.tensor.matmul(
                                y_ps[:, sb * 512:(sb + 1) * 512],
                                osT2[:, pair, ic * 128:(ic + 1) * 128],
                                wout_sb[:, pair, sb * 512:(sb + 1) * 512],
                                start=(pair == 0), stop=(pair == PAIRS - 1),
                            )
                    y_sb = ysbp.tile([128, 1024], FP, name="ysb", tag="ysb")
                    if ic % 2 == 0:
                        nc.scalar.copy(y_sb[:], y_ps[:])
                    else:
                        nc.vector.tensor_copy(y_sb[:], y_ps[:])
                    yeng = nc.sync if ic % 2 == 0 else nc.scalar
                    yeng.dma_start(y[ic * 128:(ic + 1) * 128, :], y_sb[:])

    return nc


def get_program():
    if "nc" not in _program_cache:
        _program_cache["nc"] = build_program()
    return _program_cache["nc"]


def make_in_maps(x, mask, Wqkv, Wout):
    xT_b = [np.ascontiguousarray(np.asarray(x)[b].T).astype(BF_NP) for b in range(2)]
    wq_bf = np.asarray(Wqkv).astype(BF_NP)
    wo_bf = np.asarray(Wout).astype(BF_NP)
    in_maps = []
    for c in range(8):
        b, hg = c // 4, c % 4
        ec = slice(hg * EC, (hg + 1) * EC)
        in_maps.append({
            "xT": xT_b[b],
            "wqkv": np.ascontiguousarray(wq_bf[:, ec]),
            "wout": np.ascontiguousarray(wo_bf[ec, :]),
            "mask": np.ascontiguousarray(np.asarray(mask)[b]),
        })
    return in_maps


def assemble(results, bout):
    y = np.stack([
        sum(results[b * 4 + g]["y"] for g in range(4)) for b in range(2)
    ])
    return (y + np.asarray(bout)[None, None, :]).astype(np.float32)


def kernel(x, mask, Wqkv, Wout, bout):
    _install_bir_legalizer()
    nc = get_program()
    in_maps = make_in_maps(x, mask, Wqkv, Wout)
    res = run_bass_kernel_spmd(nc, in_maps, core_ids=list(range(8)))
    return assemble(res.results, bout)


if __name__ == "__main__":
    nc = build_program()
    print("program built OK")
